# revision 1
# baseline (speedup 1.0000x reference)
# Trainium2 Bass kernel for nn_ModelPositional (gnn_message_passing).
# v8: chain-first ordering — the collective firmware needs ~67us to boot,
# so run the RWPE chain first on the PE, land both AllGather payloads
# just as the CC comes up, and bridge the remaining gather latency with
# the stage-2 pre-phase (k0..5 held across all 8 PSUM banks).

import numpy as np

B, S, KPE, V, D = 4, 512, 16, 50265, 768
NCORES = 8
VPAD = 50272          # 8 * 6284
VC = VPAD // NCORES   # 6284 vocab columns per core
DPAD = 896            # 7 * 128 (784 true dims + bias row at 784 + zero pad)
KCH = DPAD // 128     # 7 contraction chunks
P = 128
NTOK = B * S          # 2048
MT = NTOK // P        # 16 token tiles

_CACHE = {}


def _build_nc(vc=VC):
    import concourse.bacc as bacc
    import concourse.bass as bass
    import concourse.mybir as mybir
    import concourse.tile as tile
    from concourse.bass import IndirectOffsetOnAxis
    from concourse.masks import make_identity

    f32 = mybir.dt.float32
    bf16 = mybir.dt.bfloat16
    i32 = mybir.dt.int32
    Alu = mybir.AluOpType
    AX = mybir.AxisListType

    nc = bacc.Bacc(
        "TRN2",
        target_bir_lowering=False,
        debug=False,
        num_devices=NCORES,
    )

    m_in = nc.dram_tensor("m_rw", [S, S], bf16, kind="ExternalInput").ap()
    ntmT_in = nc.dram_tensor("ntmT", [S, 256], bf16, kind="ExternalInput").ap()
    codes_in = nc.dram_tensor("codes", [P, 4], i32, kind="ExternalInput").ap()
    emb_in = nc.dram_tensor("emb_table", [V, D], f32, kind="ExternalInput").ap()
    w_in = nc.dram_tensor("w_pad", [DPAD, vc], bf16, kind="ExternalInput").ap()
    logit_out = nc.dram_tensor("logit", [NTOK, vc], bf16, kind="ExternalOutput").ap()

    n_full, n_rem = divmod(vc, 512)
    ntiles = [512] * n_full + ([n_rem] if n_rem else [])

    with tile.TileContext(nc) as tc:
        with (
            tc.tile_pool(name="persist", bufs=1) as persist,
            tc.tile_pool(name="dram", bufs=1, space="DRAM") as dram,
        ):
            xTg_a = persist.tile([P, NCORES * 6, 256], bf16, name="xTg_a")
            xTg_b = persist.tile([P, NCORES, 256], bf16, name="xTg_b")
            xT_a = persist.tile([P, 6, 256], bf16, name="xT_a")
            xT_b = persist.tile([P, 1, 256], bf16, name="xT_b")
            w_all = persist.tile([P, KCH, vc], bf16, name="w_all")
            ident = persist.tile([P, P], f32, name="ident")

            cc_in_a = dram.tile([6 * P, 256], bf16, name="cc_in_a")
            cc_out_a = dram.tile(
                [NCORES * 6 * P, 256], bf16, name="cc_out_a", addr_space="Shared"
            )
            cc_in_b = dram.tile([P, 256], bf16, name="cc_in_b")
            cc_out_b = dram.tile(
                [NCORES * P, 256], bf16, name="cc_out_b", addr_space="Shared"
            )

            # ---------------- Stage 1 ----------------
            with (
                tc.tile_pool(name="s1", bufs=1) as s1,
                tc.tile_pool(name="s1tmp", bufs=3) as s1t,
                tc.tile_pool(name="psq", bufs=3, space="PSUM") as psq,
                tc.tile_pool(name="psa", bufs=3, space="PSUM") as psa,
                tc.tile_pool(name="pst", bufs=2, space="PSUM") as pst,
            ):
                codes_sb = s1.tile([P, 4], i32, name="codes_sb")
                nc.sync.dma_start(out=codes_sb[:], in_=codes_in)
                ntmT_sb = s1.tile([P, 4, 256], bf16, name="ntmT_sb")
                nc.sync.dma_start(
                    out=ntmT_sb[:], in_=ntmT_in.rearrange("(j p) r -> p j r", p=P)
                )
                m_sb = s1.tile([P, 4, S], bf16, name="m_sb")
                nc.sync.dma_start(
                    out=m_sb[:], in_=m_in.rearrange("(j p) s -> p j s", p=P)
                )
                # w resident: needed from ~105us; small stage-1 inputs first
                w_re = w_in.rearrange("(k p) v -> p k v", p=P)
                for k in range(KCH):
                    nc.sync.dma_start(out=w_all[:, k, :], in_=w_re[:, k, :])

                # eye blocks for Q0 init + diag extraction mask
                q0f = s1.tile([P, 4, 256], f32, name="q0f")
                nc.gpsimd.memset(q0f[:], 0.0)
                for jb in range(2):
                    nc.gpsimd.affine_select(
                        out=q0f[:, jb, :],
                        in_=q0f[:, jb, :],
                        compare_op=Alu.not_equal,
                        fill=1.0,
                        base=jb * P,
                        pattern=[[-1, 256]],
                        channel_multiplier=1,
                    )

                pe_pad = s1.tile([P, 2, P], f32, name="pe_pad")
                nc.gpsimd.memset(pe_pad[:], 0.0)
                nc.vector.memset(pe_pad[:, :, KPE : KPE + 1], 1.0)

                # gather embeddings for all 512 tokens of this core's batch
                emb_sb = s1.tile([P, 4, D], f32, name="emb_sb")
                for k in range(4):
                    nc.gpsimd.indirect_dma_start(
                        out=emb_sb[:, k, :],
                        out_offset=None,
                        in_=emb_in[:, :],
                        in_offset=IndirectOffsetOnAxis(
                            ap=codes_sb[:, k : k + 1], axis=0
                        ),
                    )
                make_identity(nc, ident[:])

                # ---- RWPE chain FIRST on the PE (CC boot ~67us anyway) ----
                qA = s1.tile([P, 4, 256], bf16, name="qA")
                qB = s1.tile([P, 4, 256], bf16, name="qB")
                nc.vector.tensor_copy(out=qA[:], in_=q0f[:])

                cur = qA
                for t in range(KPE):
                    nxt = qB if cur is qA else qA
                    for i in range(4):
                        pq = psq.tile([P, 256], f32, tag="pq")
                        for j in range(4):
                            nc.tensor.matmul(
                                out=pq[:],
                                lhsT=m_sb[:, j, i * P : (i + 1) * P],
                                rhs=cur[:, j, :],
                                start=(j == 0),
                                stop=(j == 3),
                            )
                        if i < 2:
                            dummy = s1t.tile([P, 256], f32, tag="ttr_dummy")
                            nc.vector.tensor_mul(dummy[:], pq[:], q0f[:, i, 0:256])
                            nc.vector.reduce_sum(
                                out=pe_pad[:, i, t : t + 1], in_=dummy[:], axis=AX.X
                            )
                        if i == 0:
                            nc.vector.tensor_copy(out=nxt[:, i, :], in_=pq[:])
                        else:
                            nc.scalar.copy(out=nxt[:, i, :], in_=pq[:])
                    cur = nxt

                # pe chunk: transpose [tokens, pe] -> [pe, tokens]
                for lj in range(2):
                    pt = pst.tile([P, P], f32, tag="pt")
                    nc.tensor.transpose(
                        out=pt[:], in_=pe_pad[:, lj, :], identity=ident[:]
                    )
                    nc.vector.tensor_copy(
                        out=xT_b[:, 0, lj * P : (lj + 1) * P], in_=pt[:]
                    )
                nc.sync.dma_start(out=cc_in_b[:, :], in_=xT_b[:, 0, :])

                # ---- emb cast + xT emb chunks (after chain on PE) ----
                emb_bf = s1.tile([P, 4, D], bf16, name="emb_bf")
                for k in range(4):
                    if k % 2 == 0:
                        nc.scalar.copy(out=emb_bf[:, k, :], in_=emb_sb[:, k, :])
                    else:
                        nc.vector.tensor_copy(out=emb_bf[:, k, :], in_=emb_sb[:, k, :])

                for w0 in (0, 3):
                    pas = {}
                    for j in range(4):
                        for c in range(w0, w0 + 3):
                            if j == 0:
                                pas[c] = psa.tile(
                                    [P, 256], f32, tag="pa", name=f"pa{c}"
                                )
                            nc.tensor.matmul(
                                out=pas[c][:],
                                lhsT=emb_bf[:, j, c * P : (c + 1) * P],
                                rhs=ntmT_sb[:, j, :],
                                start=(j == 0),
                                stop=(j == 3),
                            )
                    for c in range(w0, w0 + 3):
                        if c % 2 == 0:
                            nc.scalar.copy(out=xT_a[:, c, :], in_=pas[c][:])
                        else:
                            nc.vector.tensor_copy(out=xT_a[:, c, :], in_=pas[c][:])
                nc.sync.dma_start(
                    out=cc_in_a.rearrange("(k p) r -> p k r", p=P), in_=xT_a[:]
                )

                # ---- AllGathers (A = emb chunks first, B = pe chunk) ----
                nc.gpsimd.collective_compute(
                    "AllGather",
                    mybir.AluOpType.bypass,
                    replica_groups=[list(range(NCORES))],
                    ins=[cc_in_a[:].opt()],
                    outs=[cc_out_a[:].opt()],
                )
                nc.gpsimd.collective_compute(
                    "AllGather",
                    mybir.AluOpType.bypass,
                    replica_groups=[list(range(NCORES))],
                    ins=[cc_in_b[:].opt()],
                    outs=[cc_out_b[:].opt()],
                )
                cc_re = cc_out_a.rearrange("(ck p) r -> p ck r", p=P)
                for sl in range(8):
                    nc.sync.dma_start(
                        out=xTg_a[:, sl * 6 : (sl + 1) * 6, :],
                        in_=cc_re[:, sl * 6 : (sl + 1) * 6, :],
                    )
                nc.sync.dma_start(
                    out=xTg_b[:],
                    in_=cc_out_b.rearrange("(c p) r -> p c r", p=P),
                )

            # ---------------- Stage 2: logits = xT.T @ w ----------------
            with (
                tc.tile_pool(name="ob", bufs=2) as obp,
                tc.tile_pool(name="ps2", bufs=8, space="PSUM") as ps2,
            ):
                def lhs_for(rc, lj, k):
                    if k < 6:
                        return xTg_a[:, rc * 6 + k, lj * P : (lj + 1) * P]
                    return xTg_b[:, rc, lj * P : (lj + 1) * P]

                # pre-phase: m=0's first 8 n-tiles accumulate k0..5 into all 8
                # PSUM banks while the pe-chunk AllGather is still in flight
                held = {}
                for n in range(8):
                    po = ps2.tile([P, 512], f32, tag="po")
                    for k in range(6):
                        nc.tensor.matmul(
                            out=po[:],
                            lhsT=lhs_for(0, 0, k),
                            rhs=w_all[:, k, n * 512 : (n + 1) * 512],
                            start=(k == 0),
                            stop=False,
                        )
                    held[n] = po

                cut1, cut2 = 5 * 512, 10 * 512
                for m in range(MT):
                    rc, lj = divmod(m, 2)
                    ob = obp.tile([P, vc], bf16, tag="ob")
                    for n, ntn in enumerate(ntiles):
                        n0 = n * 512
                        if m == 0 and n < 8:
                            po = held[n]
                            nc.tensor.matmul(
                                out=po[:, 0:ntn],
                                lhsT=lhs_for(rc, lj, 6),
                                rhs=w_all[:, 6, n0 : n0 + ntn],
                                start=False,
                                stop=True,
                            )
                        else:
                            po = ps2.tile([P, 512], f32, tag="po")
                            for k in range(KCH):
                                nc.tensor.matmul(
                                    out=po[:, 0:ntn],
                                    lhsT=lhs_for(rc, lj, k),
                                    rhs=w_all[:, k, n0 : n0 + ntn],
                                    start=(k == 0),
                                    stop=(k == KCH - 1),
                                )
                        if n % 2 == 0:
                            nc.scalar.copy(out=ob[:, n0 : n0 + ntn], in_=po[:, 0:ntn])
                        else:
                            nc.vector.tensor_copy(
                                out=ob[:, n0 : n0 + ntn], in_=po[:, 0:ntn]
                            )
                        if n0 + ntn == cut1:
                            nc.sync.dma_start(
                                out=logit_out[m * P : (m + 1) * P, 0:cut1],
                                in_=ob[:, 0:cut1],
                            )
                        elif n0 + ntn == cut2:
                            nc.sync.dma_start(
                                out=logit_out[m * P : (m + 1) * P, cut1:cut2],
                                in_=ob[:, cut1:cut2],
                            )
                    nc.sync.dma_start(
                        out=logit_out[m * P : (m + 1) * P, cut2:vc],
                        in_=ob[:, cut2:vc],
                    )

    nc.compile()
    return nc


def _host_prep(code_inputs, position_idx, attn_mask, emb_table, w_lin, b_lin, vc=VC):
    import ml_dtypes

    bf = ml_dtypes.bfloat16
    code = np.asarray(code_inputs).astype(np.int32)
    pos = np.asarray(position_idx).astype(np.int32)
    attn = np.asarray(attn_mask).astype(np.float32)
    emb_t = np.ascontiguousarray(np.asarray(emb_table, dtype=np.float32))
    w = np.asarray(w_lin, dtype=np.float32)
    bias = np.asarray(b_lin, dtype=np.float32)

    w_ext = np.zeros((DPAD, NCORES * vc), np.float32)
    ncols = min(NCORES * vc, V)
    w_ext[: D + KPE, :ncols] = w[:, :ncols]
    w_ext[D + KPE, :ncols] = bias[:ncols]
    w_ext = w_ext.astype(bf)

    nodes = (pos == 0).astype(np.float32)
    token = (pos >= 2).astype(np.float32)
    eye = np.eye(S, dtype=bool)

    in_maps = []
    for c in range(NCORES):
        b, h = divmod(c, 2)
        if h == 0:
            perm = np.arange(S)
        else:
            perm = np.r_[256:512, 0:256]
        a_p = attn[b][perm][:, perm]
        tok_p = token[b][perm]
        nod_p = nodes[b][perm]

        A = np.where(eye, 1.0, a_p).astype(np.float32)
        m_rw = A / A.sum(1)[:, None]

        rowsum = (a_p[:256] * tok_p[None, :]).sum(1)
        alpha = nod_p[:256] / (rowsum + 1e-10)
        ntmT = a_p[:256].T * tok_p[:, None] * alpha[None, :]
        ntmT[:256][np.eye(256, dtype=bool)] += 1.0 - nod_p[:256]

        in_maps.append(
            {
                "m_rw": np.ascontiguousarray(m_rw.astype(bf)),
                "ntmT": np.ascontiguousarray(ntmT.astype(bf)),
                "codes": np.ascontiguousarray(code[b][perm].reshape(4, P).T),
                "emb_table": emb_t,
                "w_pad": np.ascontiguousarray(w_ext[:, c * vc : (c + 1) * vc]),
            }
        )
    return in_maps


def run(inputs, trace=False, vc=VC, **run_kwargs):
    from concourse.bass_utils import run_bass_kernel_spmd

    key = ("nc", vc)
    nc = _CACHE.get(key)
    if nc is None:
        nc = _build_nc(vc=vc)
        _CACHE[key] = nc
    in_maps = _host_prep(**inputs, vc=vc)
    res = run_bass_kernel_spmd(
        nc, in_maps, core_ids=list(range(NCORES)), trace=trace, **run_kwargs
    )
    ncols = min(NCORES * vc, V)
    logits = np.concatenate(
        [r["logit"].astype(np.float32) for r in res.results], axis=1
    )[:, :ncols]
    return logits.reshape(B, S, ncols).astype(np.float32), res


def kernel(**inputs):
    logits, _ = run(inputs, trace=False)
    return logits



# revision 8
# speedup vs baseline: 1.0892x; 1.0892x over previous
# Trainium2 Bass kernel for nn_ModelPositional (gnn_message_passing).
# v9: token-sharded stage 2 — no collectives at all. Each core computes
# stage-1 x for its own 256 tokens (RWPE chain + masked avg, as before),
# then streams the FULL output projection weight from HBM in 512-col
# tiles and emits logits for its 256 tokens x full vocab. The weight
# contraction is 6 full 128-row chunks (emb dims) + one 17-row chunk
# (16 pe dims + bias); the two K=17 matmuls per tile are row-packed
# into disjoint PE row-groups so they run concurrently.

import numpy as np

B, S, KPE, V, D = 4, 512, 16, 50265, 768
NCORES = 8
P = 128
NT = 99                 # 512-wide vocab tiles
VPADF = NT * 512        # 50688
K6 = KPE + 1            # 17 rows: pe dims + bias row
TPC = 256               # tokens per core

_CACHE = {}


def _build_nc():
    import concourse.bacc as bacc
    import concourse.mybir as mybir
    import concourse.tile as tile
    from concourse.bass import IndirectOffsetOnAxis
    from concourse.masks import make_identity

    f32 = mybir.dt.float32
    bf16 = mybir.dt.bfloat16
    i32 = mybir.dt.int32
    Alu = mybir.AluOpType
    AX = mybir.AxisListType

    nc = bacc.Bacc(
        "TRN2",
        target_bir_lowering=False,
        debug=False,
        num_devices=NCORES,
    )

    m_in = nc.dram_tensor("m_rw", [S, S], bf16, kind="ExternalInput").ap()
    ntmT_in = nc.dram_tensor("ntmT", [S, TPC], bf16, kind="ExternalInput").ap()
    codes_in = nc.dram_tensor("codes", [P, 4], i32, kind="ExternalInput").ap()
    emb_in = nc.dram_tensor("emb_table", [V, D], f32, kind="ExternalInput").ap()
    w6_in = nc.dram_tensor("w6t", [NT * P, 6 * 512], bf16, kind="ExternalInput").ap()
    w7_in = nc.dram_tensor("w7t", [NT * 64, 512], bf16, kind="ExternalInput").ap()
    logit_out = nc.dram_tensor("logit", [TPC, VPADF], bf16, kind="ExternalOutput").ap()
    logit_re = logit_out.rearrange("(m p) v -> p m v", p=P)

    with tile.TileContext(nc) as tc:
        with (
            tc.tile_pool(name="persist", bufs=1) as persist,
            tc.tile_pool(name="s1", bufs=1) as s1,
            tc.tile_pool(name="s1tmp", bufs=3) as s1t,
            tc.tile_pool(name="psq", bufs=3, space="PSUM") as psq,
            tc.tile_pool(name="pst", bufs=1, space="PSUM") as pst,
            tc.tile_pool(name="ps2", bufs=4, space="PSUM") as ps2,
            tc.tile_pool(name="wp", bufs=12) as wp,
            tc.tile_pool(name="w7p", bufs=12) as w7p,
            tc.tile_pool(name="ob", bufs=4) as obp,
        ):
            xT_a = persist.tile([P, 6, TPC], bf16, name="xT_a")
            xb2 = persist.tile([64, P], bf16, name="xb2")
            ident = persist.tile([P, P], f32, name="ident")

            # ---------------- Stage 1 ----------------
            codes_sb = s1.tile([P, 4], i32, name="codes_sb")
            nc.sync.dma_start(out=codes_sb[:], in_=codes_in)
            m_sb = s1.tile([P, 4, S], bf16, name="m_sb")
            nc.sync.dma_start(
                out=m_sb[:], in_=m_in.rearrange("(j p) s -> p j s", p=P)
            )
            ntmT_sb = s1.tile([P, 4, TPC], bf16, name="ntmT_sb")
            nc.sync.dma_start(
                out=ntmT_sb[:], in_=ntmT_in.rearrange("(j p) r -> p j r", p=P)
            )

            # PE warmup while input DMAs land: ~3us of junk matmuls on a
            # memset tile so the HAM un-throttles before the chain starts.
            warm = s1.tile([P, P], bf16, name="warm")
            nc.vector.memset(warm[:], 0.0)
            for wi in range(3):
                pw = psq.tile([P, TPC], f32, tag="pq")
                for wj in range(8):
                    nc.tensor.matmul(
                        out=pw[:, 0:P],
                        lhsT=warm[:],
                        rhs=warm[:],
                        start=(wj == 0),
                        stop=(wj == 7),
                    )

            # eye blocks for Q0 init + diag extraction mask
            q0f = s1.tile([P, 4, TPC], f32, name="q0f")
            nc.gpsimd.memset(q0f[:], 0.0)
            for jb in range(2):
                nc.gpsimd.affine_select(
                    out=q0f[:, jb, :],
                    in_=q0f[:, jb, :],
                    compare_op=Alu.not_equal,
                    fill=1.0,
                    base=jb * P,
                    pattern=[[-1, TPC]],
                    channel_multiplier=1,
                )

            # gather embeddings for all 512 tokens of this core's graph
            emb_sb = s1.tile([P, 4, D], f32, name="emb_sb")
            for k in range(4):
                nc.gpsimd.indirect_dma_start(
                    out=emb_sb[:, k, :],
                    out_offset=None,
                    in_=emb_in[:, :],
                    in_offset=IndirectOffsetOnAxis(
                        ap=codes_sb[:, k : k + 1], axis=0
                    ),
                )

            # pe_pad cols 0..16 = m0 (pe dims + ones), cols 32..48 = m1
            pe_pad = s1.tile([P, 64], f32, name="pe_pad")
            nc.gpsimd.memset(pe_pad[:], 0.0)
            nc.vector.memset(pe_pad[:, KPE : KPE + 1], 1.0)
            nc.vector.memset(pe_pad[:, 32 + KPE : 32 + KPE + 1], 1.0)
            make_identity(nc, ident[:])

            # ---- RWPE chain on the PE ----
            qA = s1.tile([P, 4, TPC], bf16, name="qA")
            qB = s1.tile([P, 4, TPC], bf16, name="qB")
            nc.vector.tensor_copy(out=qA[:], in_=q0f[:])

            cur = qA
            for t in range(KPE):
                nxt = qB if cur is qA else qA
                for i in range(4):
                    pq = psq.tile([P, TPC], f32, tag="pq")
                    for j in range(4):
                        nc.tensor.matmul(
                            out=pq[:],
                            lhsT=m_sb[:, j, i * P : (i + 1) * P],
                            rhs=cur[:, j, :],
                            start=(j == 0),
                            stop=(j == 3),
                        )
                    if i < 2:
                        dummy = s1t.tile([P, TPC], f32, tag="ttr_dummy")
                        nc.vector.tensor_mul(dummy[:], pq[:], q0f[:, i, 0:TPC])
                        nc.vector.reduce_sum(
                            out=pe_pad[:, i * 32 + t : i * 32 + t + 1],
                            in_=dummy[:],
                            axis=AX.X,
                        )
                    if i % 2 == 0:
                        nc.scalar.copy(out=nxt[:, i, :], in_=pq[:])
                    else:
                        nc.vector.tensor_copy(out=nxt[:, i, :], in_=pq[:])
                cur = nxt

            # pe chunk: one transpose [tokens, 64] -> [64, tokens]; rows
            # 0..16 = m0 pe+ones, rows 32..48 = m1 (both row-groups of xb2)
            pt = pst.tile([64, P], f32, name="pt")
            nc.tensor.transpose(out=pt[:], in_=pe_pad[:], identity=ident[:])
            nc.scalar.copy(out=xb2[0:K6, :], in_=pt[0:K6, :])
            nc.vector.tensor_copy(out=xb2[32 : 32 + K6, :], in_=pt[32 : 32 + K6, :])

            # ---- emb cast + masked-average into xT_a chunks ----
            emb_bf = s1.tile([P, 4, D], bf16, name="emb_bf")
            for k in range(4):
                if k % 2 == 0:
                    nc.scalar.copy(out=emb_bf[:, k, :], in_=emb_sb[:, k, :])
                else:
                    nc.vector.tensor_copy(out=emb_bf[:, k, :], in_=emb_sb[:, k, :])

            for w0 in (0, 3):
                pas = {}
                for j in range(4):
                    for c in range(w0, w0 + 3):
                        if j == 0:
                            pas[c] = ps2.tile([P, 512], f32, tag="po", name=f"pa{c}")
                        nc.tensor.matmul(
                            out=pas[c][:, 0:TPC],
                            lhsT=emb_bf[:, j, c * P : (c + 1) * P],
                            rhs=ntmT_sb[:, j, :],
                            start=(j == 0),
                            stop=(j == 3),
                        )
                for c in range(w0, w0 + 3):
                    if c % 2 == 0:
                        nc.scalar.copy(out=xT_a[:, c, :], in_=pas[c][:, 0:TPC])
                    else:
                        nc.vector.tensor_copy(out=xT_a[:, c, :], in_=pas[c][:, 0:TPC])

            # ---------------- Stage 2: logits = x @ w, streamed ----------------
            for n in range(NT):
                wt = wp.tile([P, 6 * 512], bf16, tag="w6")
                nc.sync.dma_start(out=wt[:], in_=w6_in[n * P : (n + 1) * P, :])
                w7s = w7p.tile([64, 512], bf16, tag="w7")
                nc.sync.dma_start(
                    out=w7s[0:49, :], in_=w7_in[n * 64 : n * 64 + 49, :]
                )

                poA = ps2.tile([P, 512], f32, tag="po")
                poB = ps2.tile([P, 512], f32, tag="po")
                for k in range(6):
                    nc.tensor.matmul(
                        out=poA[:],
                        lhsT=xT_a[:, k, 0:P],
                        rhs=wt[:, k * 512 : (k + 1) * 512],
                        start=(k == 0),
                        stop=False,
                    )
                for k in range(6):
                    nc.tensor.matmul(
                        out=poB[:],
                        lhsT=xT_a[:, k, P:TPC],
                        rhs=wt[:, k * 512 : (k + 1) * 512],
                        start=(k == 0),
                        stop=False,
                    )
                # K=17 pe+bias chunk, row-packed: row-groups 0 and 1 concurrent
                nc.tensor.matmul(
                    out=poA[:],
                    lhsT=xb2[0:K6, :],
                    rhs=w7s[0:K6, :],
                    start=False,
                    stop=True,
                )
                nc.tensor.matmul(
                    out=poB[:],
                    lhsT=xb2[32 : 32 + K6, :],
                    rhs=w7s[32 : 32 + K6, :],
                    start=False,
                    stop=True,
                )

                ob = obp.tile([P, 2, 512], bf16, tag="ob")
                nc.scalar.copy(out=ob[:, 0, :], in_=poA[:])
                nc.vector.tensor_copy(out=ob[:, 1, :], in_=poB[:])
                nc.sync.dma_start(
                    out=logit_re[:, :, n * 512 : (n + 1) * 512], in_=ob[:]
                )

    nc.compile()
    return nc


def _host_prep(code_inputs, position_idx, attn_mask, emb_table, w_lin, b_lin):
    import ml_dtypes

    bf = ml_dtypes.bfloat16
    code = np.asarray(code_inputs).astype(np.int32)
    pos = np.asarray(position_idx).astype(np.int32)
    attn = np.asarray(attn_mask).astype(np.float32)
    emb_t = np.ascontiguousarray(np.asarray(emb_table, dtype=np.float32))
    w = np.asarray(w_lin, dtype=np.float32)
    bias = np.asarray(b_lin, dtype=np.float32)

    # padded projection: rows 0..767 emb dims, 768..783 pe dims, 784 bias
    wp = np.zeros((D + K6, VPADF), np.float32)
    wp[: D + KPE, :V] = w
    wp[D + KPE, :V] = bias
    wp = wp.astype(bf)

    w6t = np.ascontiguousarray(
        wp[:D].reshape(6, P, NT, 512).transpose(2, 1, 0, 3).reshape(NT * P, 6 * 512)
    )
    w7s = wp[D : D + K6].reshape(K6, NT, 512).transpose(1, 0, 2)
    w7t = np.zeros((NT, 64, 512), bf)
    w7t[:, 0:K6] = w7s
    w7t[:, 32 : 32 + K6] = w7s
    w7t = np.ascontiguousarray(w7t.reshape(NT * 64, 512))

    nodes = (pos == 0).astype(np.float32)
    token = (pos >= 2).astype(np.float32)
    eye = np.eye(S, dtype=bool)

    in_maps = []
    for c in range(NCORES):
        b, h = divmod(c, 2)
        if h == 0:
            perm = np.arange(S)
        else:
            perm = np.r_[256:512, 0:256]
        a_p = attn[b][perm][:, perm]
        tok_p = token[b][perm]
        nod_p = nodes[b][perm]

        A = np.where(eye, 1.0, a_p).astype(np.float32)
        m_rw = A / A.sum(1)[:, None]

        rowsum = (a_p[:TPC] * tok_p[None, :]).sum(1)
        alpha = nod_p[:TPC] / (rowsum + 1e-10)
        ntmT = a_p[:TPC].T * tok_p[:, None] * alpha[None, :]
        ntmT[:TPC][np.eye(TPC, dtype=bool)] += 1.0 - nod_p[:TPC]

        in_maps.append(
            {
                "m_rw": np.ascontiguousarray(m_rw.astype(bf)),
                "ntmT": np.ascontiguousarray(ntmT.astype(bf)),
                "codes": np.ascontiguousarray(code[b][perm].reshape(4, P).T),
                "emb_table": emb_t,
                "w6t": w6t,
                "w7t": w7t,
            }
        )
    return in_maps


def run(inputs, trace=False, **run_kwargs):
    from concourse.bass_utils import run_bass_kernel_spmd

    key = "nc_v9"
    nc = _CACHE.get(key)
    if nc is None:
        nc = _build_nc()
        _CACHE[key] = nc
    in_maps = _host_prep(**inputs)
    res = run_bass_kernel_spmd(
        nc, in_maps, core_ids=list(range(NCORES)), trace=trace, **run_kwargs
    )
    out = np.empty((B, S, V), np.float32)
    for c in range(NCORES):
        b, h = divmod(c, 2)
        out[b, h * TPC : (h + 1) * TPC, :] = (
            res.results[c]["logit"][:, :V].astype(np.float32)
        )
    return out, res


def kernel(**inputs):
    logits, _ = run(inputs, trace=False)
    return logits


# revision 10
# speedup vs baseline: 1.0904x; 1.0010x over previous
# Trainium2 Bass kernel for nn_ModelPositional (gnn_message_passing).
# v10: 2-way vocab x 4-way graph hybrid sharding, no collectives.
# Core c = (graph g = c//2, vocab half v = c%2). Each core runs stage 1
# for ALL 512 tokens of its graph (RWPE chain over the full 512 columns,
# masked avg), then streams its half of the projection weight from HBM
# in 512-col tiles: logits[512 tokens, 25600 cols]. Weight traffic per
# core is ~40MB (vs ~83MB for pure token sharding), which hides fully
# under the ~330us of matmul. The 17-row pe+bias chunk is resident in
# SBUF (loaded once) and its four K=17 matmuls per tile are row-packed
# into disjoint PE row-groups so they take ~one slot.

import numpy as np

B, S, KPE, V, D = 4, 512, 16, 50265, 768
NCORES = 8
P = 128
NTH = 50                # 512-wide vocab tiles per half
HW = NTH * 512          # 25600 padded cols per half
HSTART = (0, 25152)     # col offset of each half (25152 + 25600 >= 50265)
K6 = KPE + 1            # 17 rows: pe dims + bias row

_CACHE = {}


def _build_nc():
    import concourse.bacc as bacc
    import concourse.mybir as mybir
    import concourse.tile as tile
    from concourse.bass import IndirectOffsetOnAxis
    from concourse.masks import make_identity

    f32 = mybir.dt.float32
    bf16 = mybir.dt.bfloat16
    i32 = mybir.dt.int32
    Alu = mybir.AluOpType
    AX = mybir.AxisListType

    nc = bacc.Bacc(
        "TRN2",
        target_bir_lowering=False,
        debug=False,
        num_devices=NCORES,
    )

    m_in = nc.dram_tensor("m_rw", [S, S], bf16, kind="ExternalInput").ap()
    ntmT_in = nc.dram_tensor("ntmT", [S, S], bf16, kind="ExternalInput").ap()
    codes_in = nc.dram_tensor("codes", [P, 4], i32, kind="ExternalInput").ap()
    emb_in = nc.dram_tensor("emb_table", [V, D], f32, kind="ExternalInput").ap()
    w6_in = nc.dram_tensor("w6t", [NTH * P, 6 * 512], bf16, kind="ExternalInput").ap()
    w7_in = nc.dram_tensor("w7f", [P, HW], bf16, kind="ExternalInput").ap()
    # output: per 512-col tile, contiguous [128, 4 m-chunks, 512]
    logit_out = nc.dram_tensor(
        "logit", [NTH * P, 4 * 512], bf16, kind="ExternalOutput"
    ).ap()

    with tile.TileContext(nc) as tc:
        with (
            tc.tile_pool(name="persist", bufs=1) as persist,
            tc.tile_pool(name="wp", bufs=14) as wp,
        ):
            xT_a = persist.tile([P, 6, S], bf16, name="xT_a")
            xb2 = persist.tile([P, P], bf16, name="xb2")
            ident = persist.tile([P, P], f32, name="ident")
            w7f = persist.tile([P, HW], bf16, name="w7f")
            nc.sync.dma_start(out=w7f[:], in_=w7_in)

            # ---------------- Stage 1 ----------------
            with (
                tc.tile_pool(name="s1", bufs=1) as s1,
                tc.tile_pool(name="s1tmp", bufs=3) as s1t,
                tc.tile_pool(name="psA", bufs=4, space="PSUM") as psA,
            ):
                codes_sb = s1.tile([P, 4], i32, name="codes_sb")
                nc.sync.dma_start(out=codes_sb[:], in_=codes_in)
                m_sb = s1.tile([P, 4, S], bf16, name="m_sb")
                nc.sync.dma_start(
                    out=m_sb[:], in_=m_in.rearrange("(j p) s -> p j s", p=P)
                )
                ntmT_sb = s1.tile([P, 4, S], bf16, name="ntmT_sb")
                nc.sync.dma_start(
                    out=ntmT_sb[:], in_=ntmT_in.rearrange("(j p) r -> p j r", p=P)
                )

                # PE warmup on a memset tile while input DMAs land
                warm = s1.tile([P, P], bf16, name="warm")
                nc.vector.memset(warm[:], 0.0)
                for wi in range(3):
                    pw = psA.tile([P, 512], f32, tag="pq")
                    for wj in range(8):
                        nc.tensor.matmul(
                            out=pw[:, 0:P],
                            lhsT=warm[:],
                            rhs=warm[:],
                            start=(wj == 0),
                            stop=(wj == 7),
                        )

                # eye blocks for Q0 init + diag extraction mask
                q0f = s1.tile([P, 4, S], f32, name="q0f")
                nc.gpsimd.memset(q0f[:], 0.0)
                for jb in range(4):
                    nc.gpsimd.affine_select(
                        out=q0f[:, jb, :],
                        in_=q0f[:, jb, :],
                        compare_op=Alu.not_equal,
                        fill=1.0,
                        base=jb * P,
                        pattern=[[-1, S]],
                        channel_multiplier=1,
                    )

                # gather embeddings for all 512 tokens of this core's graph
                emb_sb = s1.tile([P, 4, D], f32, name="emb_sb")
                for k in range(4):
                    nc.gpsimd.indirect_dma_start(
                        out=emb_sb[:, k, :],
                        out_offset=None,
                        in_=emb_in[:, :],
                        in_offset=IndirectOffsetOnAxis(
                            ap=codes_sb[:, k : k + 1], axis=0
                        ),
                    )

                # pe_pad cols m*32+(0..15) = diag(M^t) for token chunk m,
                # col m*32+16 = 1.0 (bias row after transpose)
                pe_pad = s1.tile([P, P], f32, name="pe_pad")
                nc.gpsimd.memset(pe_pad[:], 0.0)
                for m in range(4):
                    nc.vector.memset(pe_pad[:, m * 32 + KPE : m * 32 + KPE + 1], 1.0)
                make_identity(nc, ident[:])

                # ---- RWPE chain over the full 512 columns ----
                qA = s1.tile([P, 4, S], bf16, name="qA")
                qB = s1.tile([P, 4, S], bf16, name="qB")
                nc.vector.tensor_copy(out=qA[:], in_=q0f[:])

                cur = qA
                for t in range(KPE):
                    nxt = qB if cur is qA else qA
                    for i in range(4):
                        pq = psA.tile([P, S], f32, tag="pq")
                        for j in range(4):
                            nc.tensor.matmul(
                                out=pq[:],
                                lhsT=m_sb[:, j, i * P : (i + 1) * P],
                                rhs=cur[:, j, :],
                                start=(j == 0),
                                stop=(j == 3),
                            )
                        dummy = s1t.tile([P, S], f32, tag="ttr_dummy")
                        nc.vector.tensor_mul(dummy[:], pq[:], q0f[:, i, :])
                        nc.vector.reduce_sum(
                            out=pe_pad[:, i * 32 + t : i * 32 + t + 1],
                            in_=dummy[:],
                            axis=AX.X,
                        )
                        if i == 1:
                            nc.vector.tensor_copy(out=nxt[:, i, :], in_=pq[:])
                        else:
                            nc.scalar.copy(out=nxt[:, i, :], in_=pq[:])
                    cur = nxt

                # pe chunk: one transpose [tokens, 4*32] -> [4*32, tokens];
                # rows m*32..m*32+16 are the K=17 lhsT for token chunk m
                pt = psA.tile([P, S], f32, tag="pq")
                nc.tensor.transpose(
                    out=pt[:, 0:P], in_=pe_pad[:], identity=ident[:]
                )
                nc.vector.tensor_copy(out=xb2[:], in_=pt[:, 0:P])

                # ---- emb cast + masked-average into xT_a chunks ----
                emb_bf = s1.tile([P, 4, D], bf16, name="emb_bf")
                for k in range(4):
                    if k % 2 == 0:
                        nc.scalar.copy(out=emb_bf[:, k, :], in_=emb_sb[:, k, :])
                    else:
                        nc.vector.tensor_copy(out=emb_bf[:, k, :], in_=emb_sb[:, k, :])

                for w0 in (0, 3):
                    pas = {}
                    for j in range(4):
                        for c in range(w0, w0 + 3):
                            if j == 0:
                                pas[c] = psA.tile([P, S], f32, tag="pq", name=f"pa{c}")
                            nc.tensor.matmul(
                                out=pas[c][:],
                                lhsT=emb_bf[:, j, c * P : (c + 1) * P],
                                rhs=ntmT_sb[:, j, :],
                                start=(j == 0),
                                stop=(j == 3),
                            )
                    for c in range(w0, w0 + 3):
                        if c % 2 == 0:
                            nc.scalar.copy(out=xT_a[:, c, :], in_=pas[c][:])
                        else:
                            nc.vector.tensor_copy(out=xT_a[:, c, :], in_=pas[c][:])

            # ---------------- Stage 2: logits = x @ w, streamed ----------------
            with (
                tc.tile_pool(name="ob", bufs=4) as obp,
                tc.tile_pool(name="ps2", bufs=8, space="PSUM") as ps2,
            ):
                for n in range(NTH):
                    wt = wp.tile([P, 6 * 512], bf16, tag="w6")
                    nc.sync.dma_start(out=wt[:], in_=w6_in[n * P : (n + 1) * P, :])

                    po = []
                    for m in range(4):
                        pom = ps2.tile([P, 512], f32, tag="po")
                        po.append(pom)
                        for k in range(6):
                            nc.tensor.matmul(
                                out=pom[:],
                                lhsT=xT_a[:, k, m * P : (m + 1) * P],
                                rhs=wt[:, k * 512 : (k + 1) * 512],
                                start=(k == 0),
                                stop=False,
                            )
                    # K=17 pe+bias chunk, 4-way row-packed (concurrent)
                    for m in range(4):
                        nc.tensor.matmul(
                            out=po[m][:],
                            lhsT=xb2[m * 32 : m * 32 + K6, :],
                            rhs=w7f[m * 32 : m * 32 + K6, n * 512 : (n + 1) * 512],
                            start=False,
                            stop=True,
                            tile_position=(m * 32, 0),
                        )

                    ob = obp.tile([P, 4, 512], bf16, tag="ob")
                    for m in range(4):
                        if m % 2 == 0:
                            nc.scalar.copy(out=ob[:, m, :], in_=po[m][:])
                        else:
                            nc.vector.tensor_copy(out=ob[:, m, :], in_=po[m][:])
                    nc.sync.dma_start(
                        out=logit_out[n * P : (n + 1) * P, :], in_=ob[:]
                    )

    nc.compile()
    return nc


def _host_prep(code_inputs, position_idx, attn_mask, emb_table, w_lin, b_lin):
    import ml_dtypes

    bf = ml_dtypes.bfloat16
    code = np.asarray(code_inputs).astype(np.int32)
    pos = np.asarray(position_idx).astype(np.int32)
    attn = np.asarray(attn_mask).astype(np.float32)
    emb_t = np.ascontiguousarray(np.asarray(emb_table, dtype=np.float32))
    w = np.asarray(w_lin, dtype=np.float32)
    bias = np.asarray(b_lin, dtype=np.float32)

    # padded projection: rows 0..767 emb dims, 768..783 pe dims, 784 bias
    VP = HSTART[1] + HW
    wp_full = np.zeros((D + K6, VP), np.float32)
    wp_full[: D + KPE, :V] = w
    wp_full[D + KPE, :V] = bias

    w6ts, w7fs = [], []
    for v in range(2):
        wph = wp_full[:, HSTART[v] : HSTART[v] + HW].astype(bf)
        w6t = np.ascontiguousarray(
            wph[:D]
            .reshape(6, P, NTH, 512)
            .transpose(2, 1, 0, 3)
            .reshape(NTH * P, 6 * 512)
        )
        w7f = np.zeros((P, HW), bf)
        for m in range(4):
            w7f[m * 32 : m * 32 + K6] = wph[D : D + K6]
        w6ts.append(w6t)
        w7fs.append(np.ascontiguousarray(w7f))

    nodes = (pos == 0).astype(np.float32)
    token = (pos >= 2).astype(np.float32)
    eye = np.eye(S, dtype=bool)

    in_maps = []
    for c in range(NCORES):
        g, v = divmod(c, 2)
        a = attn[g]
        A = np.where(eye, 1.0, a).astype(np.float32)
        m_rw = A / A.sum(1)[:, None]

        rowsum = (a * token[g][None, :]).sum(1)
        alpha = nodes[g] / (rowsum + 1e-10)
        ntmT = a.T * token[g][:, None] * alpha[None, :]
        ntmT[eye] += 1.0 - nodes[g]

        in_maps.append(
            {
                "m_rw": np.ascontiguousarray(m_rw.astype(bf)),
                "ntmT": np.ascontiguousarray(ntmT.astype(bf)),
                "codes": np.ascontiguousarray(code[g].reshape(4, P).T),
                "emb_table": emb_t,
                "w6t": w6ts[v],
                "w7f": w7fs[v],
            }
        )
    return in_maps


def run(inputs, trace=False, **run_kwargs):
    from concourse.bass_utils import run_bass_kernel_spmd

    key = "nc_v10"
    nc = _CACHE.get(key)
    if nc is None:
        nc = _build_nc()
        _CACHE[key] = nc
    in_maps = _host_prep(**inputs)
    res = run_bass_kernel_spmd(
        nc, in_maps, core_ids=list(range(NCORES)), trace=trace, **run_kwargs
    )
    out = np.empty((B, S, V), np.float32)
    for c in range(NCORES):
        g, v = divmod(c, 2)
        arr = (
            res.results[c]["logit"]
            .reshape(NTH, P, 4, 512)
            .transpose(2, 1, 0, 3)
            .reshape(S, HW)
            .astype(np.float32)
        )
        lo = HSTART[v]
        hi = min(lo + HW, V)
        if v == 0:
            out[g, :, lo : HSTART[1]] = arr[:, : HSTART[1] - lo]
        else:
            out[g, :, lo:hi] = arr[:, : hi - lo]
    return out, res


def kernel(**inputs):
    logits, _ = run(inputs, trace=False)
    return logits


# revision 12
# speedup vs baseline: 1.1991x; 1.0997x over previous
# Trainium2 Bass kernel for nn_ModelPositional (gnn_message_passing).
# v10: 2-way vocab x 4-way graph hybrid sharding, no collectives.
# Core c = (graph g = c//2, vocab half v = c%2). Each core runs stage 1
# for ALL 512 tokens of its graph (RWPE chain over the full 512 columns,
# masked avg), then streams its half of the projection weight from HBM
# in 512-col tiles: logits[512 tokens, 25600 cols]. Weight traffic per
# core is ~40MB (vs ~83MB for pure token sharding), which hides fully
# under the ~330us of matmul. The 17-row pe+bias chunk is resident in
# SBUF (loaded once) and its four K=17 matmuls per tile are row-packed
# into disjoint PE row-groups so they take ~one slot.

import numpy as np

B, S, KPE, V, D = 4, 512, 16, 50265, 768
NCORES = 8
P = 128
NTH = 50                # 512-wide vocab tiles per half
HW = NTH * 512          # 25600 padded cols per half
HSTART = (0, 25152)     # col offset of each half (25152 + 25600 >= 50265)
K6 = KPE + 1            # 17 rows: pe dims + bias row

_CACHE = {}


def _build_nc():
    import concourse.bacc as bacc
    import concourse.mybir as mybir
    import concourse.tile as tile
    from concourse.bass import IndirectOffsetOnAxis
    from concourse.masks import make_identity

    f32 = mybir.dt.float32
    bf16 = mybir.dt.bfloat16
    i32 = mybir.dt.int32
    Alu = mybir.AluOpType
    AX = mybir.AxisListType

    nc = bacc.Bacc(
        "TRN2",
        target_bir_lowering=False,
        debug=False,
        num_devices=NCORES,
    )

    m_in = nc.dram_tensor("m_rw", [S, S], bf16, kind="ExternalInput").ap()
    ntmT_in = nc.dram_tensor("ntmT", [S, S], bf16, kind="ExternalInput").ap()
    codes_in = nc.dram_tensor("codes", [P, 4], i32, kind="ExternalInput").ap()
    emb_in = nc.dram_tensor("emb_table", [V, D], f32, kind="ExternalInput").ap()
    w6_in = nc.dram_tensor("w6t", [NTH * P, 6 * 512], bf16, kind="ExternalInput").ap()
    w7_in = nc.dram_tensor("w7f", [P, HW], bf16, kind="ExternalInput").ap()
    # output: per 512-col tile, contiguous [128, 4 m-chunks, 512]
    logit_out = nc.dram_tensor(
        "logit", [NTH * P, 4 * 512], bf16, kind="ExternalOutput"
    ).ap()

    with tile.TileContext(nc) as tc:
        with (
            tc.tile_pool(name="persist", bufs=1) as persist,
            tc.tile_pool(name="wp", bufs=14) as wp,
        ):
            xT_a = persist.tile([P, 6, S], bf16, name="xT_a")
            xb2 = persist.tile([P, P], bf16, name="xb2")
            ident = persist.tile([P, P], f32, name="ident")
            w7f = persist.tile([P, HW], bf16, name="w7f")

            # ---------------- Stage 1 ----------------
            with (
                tc.tile_pool(name="s1", bufs=1) as s1,
                tc.tile_pool(name="s1tmp", bufs=3) as s1t,
                tc.tile_pool(name="psA", bufs=4, space="PSUM") as psA,
            ):
                codes_sb = s1.tile([P, 4], i32, name="codes_sb")
                nc.sync.dma_start(out=codes_sb[:], in_=codes_in)
                m_sb = s1.tile([P, 4, S], bf16, name="m_sb")
                nc.sync.dma_start(
                    out=m_sb[:], in_=m_in.rearrange("(j p) s -> p j s", p=P)
                )
                ntmT_sb = s1.tile([P, 4, S], bf16, name="ntmT_sb")
                nc.sync.dma_start(
                    out=ntmT_sb[:], in_=ntmT_in.rearrange("(j p) r -> p j r", p=P)
                )
                # w7f is only needed by stage 2 — issue after stage-1 inputs
                # so it doesn't delay the chain's m_sb on the DMA queue
                nc.sync.dma_start(out=w7f[:], in_=w7_in)

                # PE warmup on a memset tile while input DMAs land
                warm = s1.tile([P, P], bf16, name="warm")
                nc.vector.memset(warm[:], 0.0)
                for wi in range(3):
                    pw = psA.tile([P, 512], f32, tag="pq")
                    for wj in range(8):
                        nc.tensor.matmul(
                            out=pw[:, 0:P],
                            lhsT=warm[:],
                            rhs=warm[:],
                            start=(wj == 0),
                            stop=(wj == 7),
                        )

                # eye blocks for Q0 init + diag extraction mask
                q0f = s1.tile([P, 4, S], f32, name="q0f")
                nc.gpsimd.memset(q0f[:], 0.0)
                for jb in range(4):
                    nc.gpsimd.affine_select(
                        out=q0f[:, jb, :],
                        in_=q0f[:, jb, :],
                        compare_op=Alu.not_equal,
                        fill=1.0,
                        base=jb * P,
                        pattern=[[-1, S]],
                        channel_multiplier=1,
                    )

                # gather embeddings for all 512 tokens of this core's graph
                emb_sb = s1.tile([P, 4, D], f32, name="emb_sb")
                for k in range(4):
                    nc.gpsimd.indirect_dma_start(
                        out=emb_sb[:, k, :],
                        out_offset=None,
                        in_=emb_in[:, :],
                        in_offset=IndirectOffsetOnAxis(
                            ap=codes_sb[:, k : k + 1], axis=0
                        ),
                    )

                # pe_pad cols m*32+(0..15) = diag(M^t) for token chunk m,
                # col m*32+16 = 1.0 (bias row after transpose)
                pe_pad = s1.tile([P, P], f32, name="pe_pad")
                nc.gpsimd.memset(pe_pad[:], 0.0)
                for m in range(4):
                    nc.vector.memset(pe_pad[:, m * 32 + KPE : m * 32 + KPE + 1], 1.0)
                make_identity(nc, ident[:])

                # ---- RWPE chain over the full 512 columns ----
                qA = s1.tile([P, 4, S], bf16, name="qA")
                qB = s1.tile([P, 4, S], bf16, name="qB")
                nc.vector.tensor_copy(out=qA[:], in_=q0f[:])

                cur = qA
                for t in range(KPE):
                    nxt = qB if cur is qA else qA
                    for i in range(4):
                        pq = psA.tile([P, S], f32, tag="pq")
                        for j in range(4):
                            nc.tensor.matmul(
                                out=pq[:],
                                lhsT=m_sb[:, j, i * P : (i + 1) * P],
                                rhs=cur[:, j, :],
                                start=(j == 0),
                                stop=(j == 3),
                            )
                        # diag lives only in the [128,128] block i of pq
                        dummy = s1t.tile([P, P], f32, tag="ttr_dummy")
                        nc.vector.tensor_mul(
                            dummy[:],
                            pq[:, i * P : (i + 1) * P],
                            q0f[:, i, i * P : (i + 1) * P],
                        )
                        nc.vector.reduce_sum(
                            out=pe_pad[:, i * 32 + t : i * 32 + t + 1],
                            in_=dummy[:],
                            axis=AX.X,
                        )
                        if i == 1:
                            nc.vector.tensor_copy(out=nxt[:, i, :], in_=pq[:])
                        else:
                            nc.scalar.copy(out=nxt[:, i, :], in_=pq[:])
                    cur = nxt

                # pe chunk: one transpose [tokens, 4*32] -> [4*32, tokens];
                # rows m*32..m*32+16 are the K=17 lhsT for token chunk m
                pt = psA.tile([P, S], f32, tag="pq")
                nc.tensor.transpose(
                    out=pt[:, 0:P], in_=pe_pad[:], identity=ident[:]
                )
                nc.vector.tensor_copy(out=xb2[:], in_=pt[:, 0:P])

                # ---- emb cast + masked-average into xT_a chunks ----
                emb_bf = s1.tile([P, 4, D], bf16, name="emb_bf")
                for k in range(4):
                    if k % 2 == 0:
                        nc.scalar.copy(out=emb_bf[:, k, :], in_=emb_sb[:, k, :])
                    else:
                        nc.vector.tensor_copy(out=emb_bf[:, k, :], in_=emb_sb[:, k, :])

                for w0 in (0, 3):
                    pas = {}
                    for j in range(4):
                        for c in range(w0, w0 + 3):
                            if j == 0:
                                pas[c] = psA.tile([P, S], f32, tag="pq", name=f"pa{c}")
                            nc.tensor.matmul(
                                out=pas[c][:],
                                lhsT=emb_bf[:, j, c * P : (c + 1) * P],
                                rhs=ntmT_sb[:, j, :],
                                start=(j == 0),
                                stop=(j == 3),
                            )
                    for c in range(w0, w0 + 3):
                        if c % 2 == 0:
                            nc.scalar.copy(out=xT_a[:, c, :], in_=pas[c][:])
                        else:
                            nc.vector.tensor_copy(out=xT_a[:, c, :], in_=pas[c][:])

            # ---------------- Stage 2: logits = x @ w, streamed ----------------
            with (
                tc.tile_pool(name="ob", bufs=4) as obp,
                tc.tile_pool(name="ps2", bufs=8, space="PSUM") as ps2,
            ):
                for n in range(NTH):
                    wt = wp.tile([P, 6 * 512], bf16, tag="w6")
                    nc.sync.dma_start(out=wt[:], in_=w6_in[n * P : (n + 1) * P, :])

                    po = []
                    for m in range(4):
                        pom = ps2.tile([P, 512], f32, tag="po")
                        po.append(pom)
                        for k in range(6):
                            nc.tensor.matmul(
                                out=pom[:],
                                lhsT=xT_a[:, k, m * P : (m + 1) * P],
                                rhs=wt[:, k * 512 : (k + 1) * 512],
                                start=(k == 0),
                                stop=False,
                            )
                    # K=17 pe+bias chunk, 4-way row-packed (concurrent)
                    for m in range(4):
                        nc.tensor.matmul(
                            out=po[m][:],
                            lhsT=xb2[m * 32 : m * 32 + K6, :],
                            rhs=w7f[m * 32 : m * 32 + K6, n * 512 : (n + 1) * 512],
                            start=False,
                            stop=True,
                            tile_position=(m * 32, 0),
                        )

                    ob = obp.tile([P, 4, 512], bf16, tag="ob")
                    for m in range(4):
                        if m % 2 == 0:
                            nc.scalar.copy(out=ob[:, m, :], in_=po[m][:])
                        else:
                            nc.vector.tensor_copy(out=ob[:, m, :], in_=po[m][:])
                    nc.sync.dma_start(
                        out=logit_out[n * P : (n + 1) * P, :], in_=ob[:]
                    )

    nc.compile()
    return nc


def _host_prep(code_inputs, position_idx, attn_mask, emb_table, w_lin, b_lin):
    import ml_dtypes

    bf = ml_dtypes.bfloat16
    code = np.asarray(code_inputs).astype(np.int32)
    pos = np.asarray(position_idx).astype(np.int32)
    attn = np.asarray(attn_mask).astype(np.float32)
    emb_t = np.ascontiguousarray(np.asarray(emb_table, dtype=np.float32))
    w = np.asarray(w_lin, dtype=np.float32)
    bias = np.asarray(b_lin, dtype=np.float32)

    # padded projection: rows 0..767 emb dims, 768..783 pe dims, 784 bias
    VP = HSTART[1] + HW
    wp_full = np.zeros((D + K6, VP), np.float32)
    wp_full[: D + KPE, :V] = w
    wp_full[D + KPE, :V] = bias

    w6ts, w7fs = [], []
    for v in range(2):
        wph = wp_full[:, HSTART[v] : HSTART[v] + HW].astype(bf)
        w6t = np.ascontiguousarray(
            wph[:D]
            .reshape(6, P, NTH, 512)
            .transpose(2, 1, 0, 3)
            .reshape(NTH * P, 6 * 512)
        )
        w7f = np.zeros((P, HW), bf)
        for m in range(4):
            w7f[m * 32 : m * 32 + K6] = wph[D : D + K6]
        w6ts.append(w6t)
        w7fs.append(np.ascontiguousarray(w7f))

    nodes = (pos == 0).astype(np.float32)
    token = (pos >= 2).astype(np.float32)
    eye = np.eye(S, dtype=bool)

    in_maps = []
    for c in range(NCORES):
        g, v = divmod(c, 2)
        a = attn[g]
        A = np.where(eye, 1.0, a).astype(np.float32)
        m_rw = A / A.sum(1)[:, None]

        rowsum = (a * token[g][None, :]).sum(1)
        alpha = nodes[g] / (rowsum + 1e-10)
        ntmT = a.T * token[g][:, None] * alpha[None, :]
        ntmT[eye] += 1.0 - nodes[g]

        in_maps.append(
            {
                "m_rw": np.ascontiguousarray(m_rw.astype(bf)),
                "ntmT": np.ascontiguousarray(ntmT.astype(bf)),
                "codes": np.ascontiguousarray(code[g].reshape(4, P).T),
                "emb_table": emb_t,
                "w6t": w6ts[v],
                "w7f": w7fs[v],
            }
        )
    return in_maps


def run(inputs, trace=False, **run_kwargs):
    from concourse.bass_utils import run_bass_kernel_spmd

    key = "nc_v10"
    nc = _CACHE.get(key)
    if nc is None:
        nc = _build_nc()
        _CACHE[key] = nc
    in_maps = _host_prep(**inputs)
    res = run_bass_kernel_spmd(
        nc, in_maps, core_ids=list(range(NCORES)), trace=trace, **run_kwargs
    )
    out = np.empty((B, S, V), np.float32)
    for c in range(NCORES):
        g, v = divmod(c, 2)
        arr = (
            res.results[c]["logit"]
            .reshape(NTH, P, 4, 512)
            .transpose(2, 1, 0, 3)
            .reshape(S, HW)
            .astype(np.float32)
        )
        lo = HSTART[v]
        hi = min(lo + HW, V)
        if v == 0:
            out[g, :, lo : HSTART[1]] = arr[:, : HSTART[1] - lo]
        else:
            out[g, :, lo:hi] = arr[:, : hi - lo]
    return out, res


def kernel(**inputs):
    logits, _ = run(inputs, trace=False)
    return logits


# revision 16
# speedup vs baseline: 1.2084x; 1.0078x over previous
# Trainium2 Bass kernel for nn_ModelPositional (gnn_message_passing).
# v10: 2-way vocab x 4-way graph hybrid sharding, no collectives.
# Core c = (graph g = c//2, vocab half v = c%2). Each core runs stage 1
# for ALL 512 tokens of its graph (RWPE chain over the full 512 columns,
# masked avg), then streams its half of the projection weight from HBM
# in 512-col tiles: logits[512 tokens, 25600 cols]. Weight traffic per
# core is ~40MB (vs ~83MB for pure token sharding), which hides fully
# under the ~330us of matmul. The 17-row pe+bias chunk is resident in
# SBUF (loaded once) and its four K=17 matmuls per tile are row-packed
# into disjoint PE row-groups so they take ~one slot.

import numpy as np

B, S, KPE, V, D = 4, 512, 16, 50265, 768
NCORES = 8
P = 128
NTH = 50                # 512-wide vocab tiles per half
HW = NTH * 512          # 25600 padded cols per half
HSTART = (0, 25152)     # col offset of each half (25152 + 25600 >= 50265)
K6 = KPE + 1            # 17 rows: pe dims + bias row

_CACHE = {}


def _build_nc():
    import concourse.bacc as bacc
    import concourse.mybir as mybir
    import concourse.tile as tile
    from concourse.bass import IndirectOffsetOnAxis
    from concourse.masks import make_identity

    f32 = mybir.dt.float32
    bf16 = mybir.dt.bfloat16
    i32 = mybir.dt.int32
    Alu = mybir.AluOpType
    AX = mybir.AxisListType

    nc = bacc.Bacc(
        "TRN2",
        target_bir_lowering=False,
        debug=False,
        num_devices=NCORES,
    )

    m_in = nc.dram_tensor("m_rw", [S, S], bf16, kind="ExternalInput").ap()
    mT_in = nc.dram_tensor("m_rwT", [S, S], bf16, kind="ExternalInput").ap()
    ntmT_in = nc.dram_tensor("ntmT", [S, S], bf16, kind="ExternalInput").ap()
    codes_in = nc.dram_tensor("codes", [P, 4], i32, kind="ExternalInput").ap()
    emb_in = nc.dram_tensor("emb_table", [V, D], f32, kind="ExternalInput").ap()
    w6_in = nc.dram_tensor("w6t", [NTH * P, 6 * 512], bf16, kind="ExternalInput").ap()
    w7_in = nc.dram_tensor("w7f", [P, HW], bf16, kind="ExternalInput").ap()
    # output: per 512-col tile, contiguous [128, 4 m-chunks, 512]
    logit_out = nc.dram_tensor(
        "logit", [NTH * P, 4 * 512], bf16, kind="ExternalOutput"
    ).ap()

    with tile.TileContext(nc) as tc:
        with (
            tc.tile_pool(name="persist", bufs=1) as persist,
            tc.tile_pool(name="wp", bufs=14) as wp,
        ):
            xT_a = persist.tile([P, 6, S], bf16, name="xT_a")
            xb2 = persist.tile([P, P], bf16, name="xb2")
            ident = persist.tile([P, P], f32, name="ident")
            w7f = persist.tile([P, HW], bf16, name="w7f")

            # ---------------- Stage 1 ----------------
            with (
                tc.tile_pool(name="s1", bufs=1) as s1,
                tc.tile_pool(name="s1tmp", bufs=3) as s1t,
                tc.tile_pool(name="psA", bufs=4, space="PSUM") as psA,
            ):
                codes_sb = s1.tile([P, 4], i32, name="codes_sb")
                nc.sync.dma_start(out=codes_sb[:], in_=codes_in)
                m_sb = s1.tile([P, 4, S], bf16, name="m_sb")
                nc.sync.dma_start(
                    out=m_sb[:], in_=m_in.rearrange("(j p) s -> p j s", p=P)
                )
                # chain state Q_1 = M^T comes straight from the host — the
                # first matmul step of the power chain is skipped entirely
                qA = s1.tile([P, 4, S], bf16, name="qA")
                qB = s1.tile([P, 4, S], bf16, name="qB")
                nc.sync.dma_start(
                    out=qA[:], in_=mT_in.rearrange("(j p) s -> p j s", p=P)
                )
                ntmT_sb = s1.tile([P, 4, S], bf16, name="ntmT_sb")
                nc.sync.dma_start(
                    out=ntmT_sb[:], in_=ntmT_in.rearrange("(j p) r -> p j r", p=P)
                )
                # w7f is only needed by stage 2 — issue after stage-1 inputs
                # so it doesn't delay the chain's m_sb on the DMA queue
                nc.sync.dma_start(out=w7f[:], in_=w7_in)

                # PE warmup on a memset tile while input DMAs land
                warm = s1.tile([P, P], bf16, name="warm")
                nc.vector.memset(warm[:], 0.0)
                for wi in range(3):
                    pw = psA.tile([P, 512], f32, tag="pq")
                    for wj in range(8):
                        nc.tensor.matmul(
                            out=pw[:, 0:P],
                            lhsT=warm[:],
                            rhs=warm[:],
                            start=(wj == 0),
                            stop=(wj == 7),
                        )

                # pe_pad cols m*32+(0..15) = diag(M^t) for token chunk m,
                # col m*32+16 = 1.0 (bias row after transpose).
                # Keep these gpsimd ops BEFORE the gathers: the chain's diag
                # writes depend on them.
                pe_pad = s1.tile([P, P], f32, name="pe_pad")
                nc.gpsimd.memset(pe_pad[:], 0.0)
                for m in range(4):
                    nc.vector.memset(pe_pad[:, m * 32 + KPE : m * 32 + KPE + 1], 1.0)
                make_identity(nc, ident[:])

                # gather embeddings for all 512 tokens of this core's graph
                emb_sb = s1.tile([P, 4, D], f32, name="emb_sb")
                for k in range(4):
                    nc.gpsimd.indirect_dma_start(
                        out=emb_sb[:, k, :],
                        out_offset=None,
                        in_=emb_in[:, :],
                        in_offset=IndirectOffsetOnAxis(
                            ap=codes_sb[:, k : k + 1], axis=0
                        ),
                    )

                # ---- RWPE chain over the full 512 columns ----
                # diag(M^1) straight from m_sb's diagonal blocks
                for i in range(4):
                    dummy = s1t.tile([P, P], f32, tag="ttr_dummy")
                    nc.vector.tensor_mul(
                        dummy[:], m_sb[:, i, i * P : (i + 1) * P], ident[:]
                    )
                    nc.vector.reduce_sum(
                        out=pe_pad[:, i * 32 : i * 32 + 1],
                        in_=dummy[:],
                        axis=AX.X,
                    )

                cur = qA
                for t in range(1, KPE):
                    nxt = qB if cur is qA else qA
                    for i in range(4):
                        pq = psA.tile([P, S], f32, tag="pq")
                        for j in range(4):
                            nc.tensor.matmul(
                                out=pq[:],
                                lhsT=m_sb[:, j, i * P : (i + 1) * P],
                                rhs=cur[:, j, :],
                                start=(j == 0),
                                stop=(j == 3),
                            )
                        # diag lives only in the [128,128] block i of pq
                        dummy = s1t.tile([P, P], f32, tag="ttr_dummy")
                        nc.vector.tensor_mul(
                            dummy[:], pq[:, i * P : (i + 1) * P], ident[:]
                        )
                        nc.vector.reduce_sum(
                            out=pe_pad[:, i * 32 + t : i * 32 + t + 1],
                            in_=dummy[:],
                            axis=AX.X,
                        )
                        if i == 1:
                            nc.vector.tensor_copy(out=nxt[:, i, :], in_=pq[:])
                        else:
                            nc.scalar.copy(out=nxt[:, i, :], in_=pq[:])
                    cur = nxt

                # pe chunk: one transpose [tokens, 4*32] -> [4*32, tokens];
                # rows m*32..m*32+16 are the K=17 lhsT for token chunk m
                pt = psA.tile([P, S], f32, tag="pq")
                nc.tensor.transpose(
                    out=pt[:, 0:P], in_=pe_pad[:], identity=ident[:]
                )
                nc.vector.tensor_copy(out=xb2[:], in_=pt[:, 0:P])

                # ---- emb cast + masked-average into xT_a chunks ----
                emb_bf = s1.tile([P, 4, D], bf16, name="emb_bf")
                for k in range(4):
                    if k % 2 == 0:
                        nc.scalar.copy(out=emb_bf[:, k, :], in_=emb_sb[:, k, :])
                    else:
                        nc.vector.tensor_copy(out=emb_bf[:, k, :], in_=emb_sb[:, k, :])

                for w0 in (0, 3):
                    pas = {}
                    for j in range(4):
                        for c in range(w0, w0 + 3):
                            if j == 0:
                                pas[c] = psA.tile([P, S], f32, tag="pq", name=f"pa{c}")
                            nc.tensor.matmul(
                                out=pas[c][:],
                                lhsT=emb_bf[:, j, c * P : (c + 1) * P],
                                rhs=ntmT_sb[:, j, :],
                                start=(j == 0),
                                stop=(j == 3),
                            )
                    for c in range(w0, w0 + 3):
                        if c % 2 == 0:
                            nc.scalar.copy(out=xT_a[:, c, :], in_=pas[c][:])
                        else:
                            nc.vector.tensor_copy(out=xT_a[:, c, :], in_=pas[c][:])

            # ---------------- Stage 2: logits = x @ w, streamed ----------------
            with (
                tc.tile_pool(name="ob", bufs=4) as obp,
                tc.tile_pool(name="ps2", bufs=8, space="PSUM") as ps2,
            ):
                for n in range(NTH):
                    wt = wp.tile([P, 6 * 512], bf16, tag="w6")
                    nc.sync.dma_start(out=wt[:], in_=w6_in[n * P : (n + 1) * P, :])

                    po = []
                    for m in range(4):
                        pom = ps2.tile([P, 512], f32, tag="po")
                        po.append(pom)
                        for k in range(6):
                            nc.tensor.matmul(
                                out=pom[:],
                                lhsT=xT_a[:, k, m * P : (m + 1) * P],
                                rhs=wt[:, k * 512 : (k + 1) * 512],
                                start=(k == 0),
                                stop=False,
                            )
                    # K=17 pe+bias chunk, 4-way row-packed (concurrent)
                    for m in range(4):
                        nc.tensor.matmul(
                            out=po[m][:],
                            lhsT=xb2[m * 32 : m * 32 + K6, :],
                            rhs=w7f[m * 32 : m * 32 + K6, n * 512 : (n + 1) * 512],
                            start=False,
                            stop=True,
                            tile_position=(m * 32, 0),
                        )

                    ob = obp.tile([P, 4, 512], bf16, tag="ob")
                    for m in range(4):
                        if m % 2 == 0:
                            nc.scalar.copy(out=ob[:, m, :], in_=po[m][:])
                        else:
                            nc.vector.tensor_copy(out=ob[:, m, :], in_=po[m][:])
                    nc.sync.dma_start(
                        out=logit_out[n * P : (n + 1) * P, :], in_=ob[:]
                    )

    nc.compile()
    return nc


def _host_prep(code_inputs, position_idx, attn_mask, emb_table, w_lin, b_lin):
    import ml_dtypes

    bf = ml_dtypes.bfloat16
    code = np.asarray(code_inputs).astype(np.int32)
    pos = np.asarray(position_idx).astype(np.int32)
    attn = np.asarray(attn_mask).astype(np.float32)
    emb_t = np.ascontiguousarray(np.asarray(emb_table, dtype=np.float32))
    w = np.asarray(w_lin, dtype=np.float32)
    bias = np.asarray(b_lin, dtype=np.float32)

    # padded projection: rows 0..767 emb dims, 768..783 pe dims, 784 bias
    VP = HSTART[1] + HW
    wp_full = np.zeros((D + K6, VP), np.float32)
    wp_full[: D + KPE, :V] = w
    wp_full[D + KPE, :V] = bias

    w6ts, w7fs = [], []
    for v in range(2):
        wph = wp_full[:, HSTART[v] : HSTART[v] + HW].astype(bf)
        w6t = np.ascontiguousarray(
            wph[:D]
            .reshape(6, P, NTH, 512)
            .transpose(2, 1, 0, 3)
            .reshape(NTH * P, 6 * 512)
        )
        w7f = np.zeros((P, HW), bf)
        for m in range(4):
            w7f[m * 32 : m * 32 + K6] = wph[D : D + K6]
        w6ts.append(w6t)
        w7fs.append(np.ascontiguousarray(w7f))

    nodes = (pos == 0).astype(np.float32)
    token = (pos >= 2).astype(np.float32)
    eye = np.eye(S, dtype=bool)

    in_maps = []
    for c in range(NCORES):
        g, v = divmod(c, 2)
        a = attn[g]
        A = np.where(eye, 1.0, a).astype(np.float32)
        m_rw = A / A.sum(1)[:, None]

        rowsum = (a * token[g][None, :]).sum(1)
        alpha = nodes[g] / (rowsum + 1e-10)
        ntmT = a.T * token[g][:, None] * alpha[None, :]
        ntmT[eye] += 1.0 - nodes[g]

        m_bf = m_rw.astype(bf)
        in_maps.append(
            {
                "m_rw": np.ascontiguousarray(m_bf),
                "m_rwT": np.ascontiguousarray(m_bf.T),
                "ntmT": np.ascontiguousarray(ntmT.astype(bf)),
                "codes": np.ascontiguousarray(code[g].reshape(4, P).T),
                "emb_table": emb_t,
                "w6t": w6ts[v],
                "w7f": w7fs[v],
            }
        )
    return in_maps


def run(inputs, trace=False, **run_kwargs):
    from concourse.bass_utils import run_bass_kernel_spmd

    key = "nc_v10"
    nc = _CACHE.get(key)
    if nc is None:
        nc = _build_nc()
        _CACHE[key] = nc
    in_maps = _host_prep(**inputs)
    res = run_bass_kernel_spmd(
        nc, in_maps, core_ids=list(range(NCORES)), trace=trace, **run_kwargs
    )
    out = np.empty((B, S, V), np.float32)
    for c in range(NCORES):
        g, v = divmod(c, 2)
        arr = (
            res.results[c]["logit"]
            .reshape(NTH, P, 4, 512)
            .transpose(2, 1, 0, 3)
            .reshape(S, HW)
            .astype(np.float32)
        )
        lo = HSTART[v]
        hi = min(lo + HW, V)
        if v == 0:
            out[g, :, lo : HSTART[1]] = arr[:, : HSTART[1] - lo]
        else:
            out[g, :, lo:hi] = arr[:, : hi - lo]
    return out, res


def kernel(**inputs):
    logits, _ = run(inputs, trace=False)
    return logits


# revision 19
# speedup vs baseline: 1.2727x; 1.0532x over previous
# Trainium2 Bass kernel for nn_ModelPositional (gnn_message_passing).
# v10: 2-way vocab x 4-way graph hybrid sharding, no collectives.
# Core c = (graph g = c//2, vocab half v = c%2). Each core runs stage 1
# for ALL 512 tokens of its graph (RWPE chain over the full 512 columns,
# masked avg), then streams its half of the projection weight from HBM
# in 512-col tiles: logits[512 tokens, 25600 cols]. Weight traffic per
# core is ~40MB (vs ~83MB for pure token sharding), which hides fully
# under the ~330us of matmul. The 17-row pe+bias chunk is resident in
# SBUF (loaded once) and its four K=17 matmuls per tile are row-packed
# into disjoint PE row-groups so they take ~one slot.

import numpy as np

B, S, KPE, V, D = 4, 512, 16, 50265, 768
NCORES = 8
P = 128
NTH = 50                # 512-wide vocab tiles per half
HW = NTH * 512          # 25600 padded cols per half
HSTART = (0, 25152)     # col offset of each half (25152 + 25600 >= 50265)
K6 = KPE + 1            # 17 rows: pe dims + bias row

_CACHE = {}


def _build_nc():
    import concourse.bacc as bacc
    import concourse.mybir as mybir
    import concourse.tile as tile
    from concourse.bass import IndirectOffsetOnAxis
    from concourse.masks import make_identity

    f32 = mybir.dt.float32
    bf16 = mybir.dt.bfloat16
    i32 = mybir.dt.int32
    Alu = mybir.AluOpType
    AX = mybir.AxisListType

    nc = bacc.Bacc(
        "TRN2",
        target_bir_lowering=False,
        debug=False,
        num_devices=NCORES,
    )

    m_in = nc.dram_tensor("m_rw", [S, S], bf16, kind="ExternalInput").ap()
    mT_in = nc.dram_tensor("m_rwT", [S, S], bf16, kind="ExternalInput").ap()
    ntmT_in = nc.dram_tensor("ntmT", [S, S], bf16, kind="ExternalInput").ap()
    codes_in = nc.dram_tensor("codes", [P, 4], i32, kind="ExternalInput").ap()
    emb_in = nc.dram_tensor("emb_table", [V, D], f32, kind="ExternalInput").ap()
    w6_in = nc.dram_tensor("w6t", [NTH * P, 6 * 512], bf16, kind="ExternalInput").ap()
    w7_in = nc.dram_tensor("w7f", [P, HW], bf16, kind="ExternalInput").ap()
    # output: per 512-col tile, contiguous [128, 4 m-chunks, 512]
    logit_out = nc.dram_tensor(
        "logit", [NTH * P, 4 * 512], bf16, kind="ExternalOutput"
    ).ap()

    with tile.TileContext(nc) as tc:
        with (
            tc.tile_pool(name="persist", bufs=1) as persist,
            tc.tile_pool(name="wp", bufs=14) as wp,
        ):
            xT_a = persist.tile([P, 6, S], bf16, name="xT_a")
            xb2 = persist.tile([P, P], bf16, name="xb2")
            ident = persist.tile([P, P], f32, name="ident")
            w7f = persist.tile([P, HW], bf16, name="w7f")

            # ---------------- Stage 1 ----------------
            with (
                tc.tile_pool(name="s1", bufs=1) as s1,
                tc.tile_pool(name="s1tmp", bufs=3) as s1t,
                tc.tile_pool(name="psA", bufs=6, space="PSUM") as psA,
            ):
                codes_sb = s1.tile([P, 4], i32, name="codes_sb")
                nc.sync.dma_start(out=codes_sb[:], in_=codes_in)
                m_sb = s1.tile([P, 4, S], bf16, name="m_sb")
                nc.sync.dma_start(
                    out=m_sb[:], in_=m_in.rearrange("(j p) s -> p j s", p=P)
                )
                # chain state Q_1 = M^T comes straight from the host — the
                # first matmul step of the power chain is skipped entirely
                qA = s1.tile([P, 4, S], bf16, name="qA")
                qB = s1.tile([P, 4, S], bf16, name="qB")
                nc.sync.dma_start(
                    out=qA[:], in_=mT_in.rearrange("(j p) s -> p j s", p=P)
                )
                ntmT_sb = s1.tile([P, 4, S], bf16, name="ntmT_sb")
                nc.sync.dma_start(
                    out=ntmT_sb[:], in_=ntmT_in.rearrange("(j p) r -> p j r", p=P)
                )
                # w7f is only needed by stage 2 — issue after stage-1 inputs
                # so it doesn't delay the chain's m_sb on the DMA queue
                nc.sync.dma_start(out=w7f[:], in_=w7_in)

                # PE warmup right before the chain: depends on m_sb so it
                # starts when the DMA lands and hands off into the chain
                # with the HAM already (or nearly) un-throttled
                for wi in range(2):
                    pw = psA.tile([P, 512], f32, tag="pq")
                    for wj in range(8):
                        nc.tensor.matmul(
                            out=pw[:, 0:P],
                            lhsT=m_sb[:, 0, 0:P],
                            rhs=m_sb[:, 0, 0:P],
                            start=(wj == 0),
                            stop=(wj == 7),
                        )

                # pe_pad cols m*32+(0..15) = diag(M^t) for token chunk m,
                # col m*32+16 = 1.0 (bias row after transpose).
                # Keep these gpsimd ops BEFORE the gathers: the chain's diag
                # writes depend on them.
                pe_pad = s1.tile([P, P], f32, name="pe_pad")
                nc.gpsimd.memset(pe_pad[:], 0.0)
                for m in range(4):
                    nc.vector.memset(pe_pad[:, m * 32 + KPE : m * 32 + KPE + 1], 1.0)
                make_identity(nc, ident[:])

                # gather embeddings for all 512 tokens of this core's graph
                emb_sb = s1.tile([P, 4, D], f32, name="emb_sb")
                for k in range(4):
                    nc.gpsimd.indirect_dma_start(
                        out=emb_sb[:, k, :],
                        out_offset=None,
                        in_=emb_in[:, :],
                        in_offset=IndirectOffsetOnAxis(
                            ap=codes_sb[:, k : k + 1], axis=0
                        ),
                    )

                # ---- RWPE chain over the full 512 columns ----
                # diag(M^1) straight from m_sb's diagonal blocks
                for i in range(4):
                    dummy = s1t.tile([P, P], f32, tag="ttr_dummy")
                    nc.vector.tensor_mul(
                        dummy[:], m_sb[:, i, i * P : (i + 1) * P], ident[:]
                    )
                    nc.vector.reduce_sum(
                        out=pe_pad[:, i * 32 : i * 32 + 1],
                        in_=dummy[:],
                        axis=AX.X,
                    )

                cur = qA
                for t in range(1, KPE):
                    nxt = qB if cur is qA else qA
                    for i in range(4):
                        pq = psA.tile([P, S], f32, tag="pq")
                        for j in range(4):
                            nc.tensor.matmul(
                                out=pq[:],
                                lhsT=m_sb[:, j, i * P : (i + 1) * P],
                                rhs=cur[:, j, :],
                                start=(j == 0),
                                stop=(j == 3),
                            )
                        # diag lives only in the [128,128] block i of pq
                        dummy = s1t.tile([P, P], f32, tag="ttr_dummy")
                        nc.vector.tensor_mul(
                            dummy[:], pq[:, i * P : (i + 1) * P], ident[:]
                        )
                        nc.vector.reduce_sum(
                            out=pe_pad[:, i * 32 + t : i * 32 + t + 1],
                            in_=dummy[:],
                            axis=AX.X,
                        )
                        if i == 1:
                            nc.vector.tensor_copy(out=nxt[:, i, :], in_=pq[:])
                        else:
                            nc.scalar.copy(out=nxt[:, i, :], in_=pq[:])
                    cur = nxt

                # pe chunk: one transpose [tokens, 4*32] -> [4*32, tokens];
                # rows m*32..m*32+16 are the K=17 lhsT for token chunk m
                pt = psA.tile([P, S], f32, tag="pq")
                nc.tensor.transpose(
                    out=pt[:, 0:P], in_=pe_pad[:], identity=ident[:]
                )
                nc.vector.tensor_copy(out=xb2[:], in_=pt[:, 0:P])

                # ---- emb cast + masked-average into xT_a chunks ----
                # casts go on gpsimd: scalar/vector carry the chain's critical
                # copies, and the scheduler would otherwise front-load these
                # casts (which wait on the slow gather) into their streams
                emb_bf = s1.tile([P, 4, D], bf16, name="emb_bf")
                for k in range(4):
                    nc.gpsimd.tensor_copy(out=emb_bf[:, k, :], in_=emb_sb[:, k, :])

                for w0 in (0, 3):
                    pas = {}
                    for j in range(4):
                        for c in range(w0, w0 + 3):
                            if j == 0:
                                pas[c] = psA.tile([P, S], f32, tag="pq", name=f"pa{c}")
                            nc.tensor.matmul(
                                out=pas[c][:],
                                lhsT=emb_bf[:, j, c * P : (c + 1) * P],
                                rhs=ntmT_sb[:, j, :],
                                start=(j == 0),
                                stop=(j == 3),
                            )
                    for c in range(w0, w0 + 3):
                        if c % 2 == 0:
                            nc.scalar.copy(out=xT_a[:, c, :], in_=pas[c][:])
                        else:
                            nc.vector.tensor_copy(out=xT_a[:, c, :], in_=pas[c][:])

            # ---------------- Stage 2: logits = x @ w, streamed ----------------
            with (
                tc.tile_pool(name="ob", bufs=4) as obp,
                tc.tile_pool(name="ps2", bufs=8, space="PSUM") as ps2,
            ):
                for n in range(NTH):
                    wt = wp.tile([P, 6 * 512], bf16, tag="w6")
                    nc.sync.dma_start(out=wt[:], in_=w6_in[n * P : (n + 1) * P, :])

                    po = []
                    for m in range(4):
                        pom = ps2.tile([P, 512], f32, tag="po")
                        po.append(pom)
                        for k in range(6):
                            nc.tensor.matmul(
                                out=pom[:],
                                lhsT=xT_a[:, k, m * P : (m + 1) * P],
                                rhs=wt[:, k * 512 : (k + 1) * 512],
                                start=(k == 0),
                                stop=False,
                            )
                    # K=17 pe+bias chunk, 4-way row-packed (concurrent)
                    for m in range(4):
                        nc.tensor.matmul(
                            out=po[m][:],
                            lhsT=xb2[m * 32 : m * 32 + K6, :],
                            rhs=w7f[m * 32 : m * 32 + K6, n * 512 : (n + 1) * 512],
                            start=False,
                            stop=True,
                            tile_position=(m * 32, 0),
                        )

                    ob = obp.tile([P, 4, 512], bf16, tag="ob")
                    for m in range(4):
                        if m % 2 == 0:
                            nc.scalar.copy(out=ob[:, m, :], in_=po[m][:])
                        else:
                            nc.vector.tensor_copy(out=ob[:, m, :], in_=po[m][:])
                    nc.sync.dma_start(
                        out=logit_out[n * P : (n + 1) * P, :], in_=ob[:]
                    )

    nc.compile()
    return nc


def _host_prep(code_inputs, position_idx, attn_mask, emb_table, w_lin, b_lin):
    import ml_dtypes

    bf = ml_dtypes.bfloat16
    code = np.asarray(code_inputs).astype(np.int32)
    pos = np.asarray(position_idx).astype(np.int32)
    attn = np.asarray(attn_mask).astype(np.float32)
    emb_t = np.ascontiguousarray(np.asarray(emb_table, dtype=np.float32))
    w = np.asarray(w_lin, dtype=np.float32)
    bias = np.asarray(b_lin, dtype=np.float32)

    # padded projection: rows 0..767 emb dims, 768..783 pe dims, 784 bias
    VP = HSTART[1] + HW
    wp_full = np.zeros((D + K6, VP), np.float32)
    wp_full[: D + KPE, :V] = w
    wp_full[D + KPE, :V] = bias

    w6ts, w7fs = [], []
    for v in range(2):
        wph = wp_full[:, HSTART[v] : HSTART[v] + HW].astype(bf)
        w6t = np.ascontiguousarray(
            wph[:D]
            .reshape(6, P, NTH, 512)
            .transpose(2, 1, 0, 3)
            .reshape(NTH * P, 6 * 512)
        )
        w7f = np.zeros((P, HW), bf)
        for m in range(4):
            w7f[m * 32 : m * 32 + K6] = wph[D : D + K6]
        w6ts.append(w6t)
        w7fs.append(np.ascontiguousarray(w7f))

    nodes = (pos == 0).astype(np.float32)
    token = (pos >= 2).astype(np.float32)
    eye = np.eye(S, dtype=bool)

    in_maps = []
    for c in range(NCORES):
        g, v = divmod(c, 2)
        a = attn[g]
        A = np.where(eye, 1.0, a).astype(np.float32)
        m_rw = A / A.sum(1)[:, None]

        rowsum = (a * token[g][None, :]).sum(1)
        alpha = nodes[g] / (rowsum + 1e-10)
        ntmT = a.T * token[g][:, None] * alpha[None, :]
        ntmT[eye] += 1.0 - nodes[g]

        m_bf = m_rw.astype(bf)
        in_maps.append(
            {
                "m_rw": np.ascontiguousarray(m_bf),
                "m_rwT": np.ascontiguousarray(m_bf.T),
                "ntmT": np.ascontiguousarray(ntmT.astype(bf)),
                "codes": np.ascontiguousarray(code[g].reshape(4, P).T),
                "emb_table": emb_t,
                "w6t": w6ts[v],
                "w7f": w7fs[v],
            }
        )
    return in_maps


def run(inputs, trace=False, **run_kwargs):
    from concourse.bass_utils import run_bass_kernel_spmd

    key = "nc_v10"
    nc = _CACHE.get(key)
    if nc is None:
        nc = _build_nc()
        _CACHE[key] = nc
    in_maps = _host_prep(**inputs)
    res = run_bass_kernel_spmd(
        nc, in_maps, core_ids=list(range(NCORES)), trace=trace, **run_kwargs
    )
    out = np.empty((B, S, V), np.float32)
    for c in range(NCORES):
        g, v = divmod(c, 2)
        arr = (
            res.results[c]["logit"]
            .reshape(NTH, P, 4, 512)
            .transpose(2, 1, 0, 3)
            .reshape(S, HW)
            .astype(np.float32)
        )
        lo = HSTART[v]
        hi = min(lo + HW, V)
        if v == 0:
            out[g, :, lo : HSTART[1]] = arr[:, : HSTART[1] - lo]
        else:
            out[g, :, lo:hi] = arr[:, : hi - lo]
    return out, res


def kernel(**inputs):
    logits, _ = run(inputs, trace=False)
    return logits
